# revision 13
# baseline (speedup 1.0000x reference)
"""Trainium2 Bass kernel: 16-head attention (S=4096, D=1024) sharded 2 heads/core over 8 cores.

Layout per core c (slice = c*128:(c+1)*128 of the hidden dim = heads 2c, 2c+1):
  - host passes xT_pad [1152, 4096]  (x.T padded: row 1024 = ones for bias fold, rest 0)
  - wq/wk/wv [1152, 128]: rows 0:1024 = W[slice].T, row 1024 = b[slice]
  - wo [128, 1024] = Wo[:, slice].T
  - device computes QT,KT [128f, 4096q], V [4096k, 128d], then per 512-query block:
    scoresT[k, q] = (K Q^T), exp (scale=1/8 folded in, no max-subtraction: scores ~ N(0,1)),
    PV with an appended ones-column in V giving softmax denominators, normalization via a
    broadcast-reciprocal matmul, then partial out-projection. Host sums the 8 partials.
"""

import os
import sys

import numpy as np
import ml_dtypes

if os.path.isdir("/opt/trn_rl_repo") and "/opt/trn_rl_repo" not in sys.path:
    sys.path.insert(0, "/opt/trn_rl_repo")

from contextlib import ExitStack

from concourse import bass, tile
from concourse.bass_utils import run_bass_kernel_spmd
from concourse.masks import make_identity

mybir = bass.mybir
F32 = mybir.dt.float32
F32R = mybir.dt.float32r
BF16 = mybir.dt.bfloat16

P = 128
S = 4096
HID = 1024
HC = 1152          # padded contraction: 9 chunks of 128 (chunk 8 carries the bias fold)
NCH = 9
NCORES = 8
QB = 512           # query block
NQB = S // QB      # 8
NKT = S // P       # 32 key tiles
HD = 64            # head dim; 2 local heads per core




def _split_multiwaits(bir_json):
    """Walrus in this toolchain encodes at most one semaphore wait per TPB
    instruction; hoist extra waits onto injected pure-wait EventSemaphore
    instructions immediately before, on the same engine."""
    import json as _json

    bir = _json.loads(bir_json)
    n = [0]
    for fn in bir["functions"]:
        for blk in fn["blocks"]:
            out = []
            for ins in blk["instructions"]:
                si = ins.get("sync_info") or {}
                waits = si.get("on_wait") or []
                if len(waits) > 1 and ins.get("opcode") != "EventSemaphore":
                    for w in waits[:-1]:
                        n[0] += 1
                        out.append({
                            "debug": ins.get("debug", 0),
                            "engine": ins["engine"],
                            "ins": [],
                            "name": f"{ins['name']}_sw{n[0]}",
                            "opcode": "EventSemaphore",
                            "outs": [],
                            "sync_info": {"on_update": [], "on_wait": [w]},
                        })
                    si["on_wait"] = [waits[-1]]
                out.append(ins)
            blk["instructions"] = out
    return _json.dumps(bir).encode()


def _install_compile_patch():
    from concourse import bass_utils as _bu
    from concourse import bass2jax as _b2j

    if getattr(_bu, "_ant_waitsplit", False):
        return
    _orig = _bu.compile_bir_kernel

    def _patched(bir_json, tmpdir, neff_name="file.neff"):
        return _orig(_split_multiwaits(bir_json), tmpdir, neff_name)

    _bu.compile_bir_kernel = _patched
    _b2j.compile_bir_kernel = _patched
    _bu._ant_waitsplit = True


_install_compile_patch()


def _build_nc():
    nc = bass.Bass()
    xt_d = nc.declare_dram_parameter("xt", [HC, S], BF16, isOutput=False)
    wq_d = nc.declare_dram_parameter("wq", [HC, P], BF16, isOutput=False)
    wk_d = nc.declare_dram_parameter("wk", [HC, P], BF16, isOutput=False)
    wv_d = nc.declare_dram_parameter("wv", [HC, P], BF16, isOutput=False)
    wo_d = nc.declare_dram_parameter("wo", [P, HID], BF16, isOutput=False)
    sel2_d = nc.declare_dram_parameter("sel2", [2, P], BF16, isOutput=False)
    out_d = nc.declare_dram_parameter("out", [S, HID], F32, isOutput=True)

    with tile.TileContext(nc) as tc, ExitStack() as ctx:
        consts = ctx.enter_context(tc.tile_pool(name="consts", bufs=1))
        resident = ctx.enter_context(tc.tile_pool(name="resident", bufs=1))

        # --- constants ---
        wq_sb = consts.tile([P, NCH, P], BF16, tag="wq")
        wk_sb = consts.tile([P, NCH, P], BF16, tag="wk")
        wv_sb = consts.tile([P, NCH, P], BF16, tag="wv")
        nc.sync.dma_start(wq_sb[:], wq_d.rearrange("(c p) m -> p c m", p=P))
        nc.sync.dma_start(wk_sb[:], wk_d.rearrange("(c p) m -> p c m", p=P))
        nc.sync.dma_start(wv_sb[:], wv_d.rearrange("(c p) m -> p c m", p=P))
        wo_sb = consts.tile([P, HID], BF16, tag="wo")
        nc.sync.dma_start(wo_sb[:], wo_d[:])
        ident = consts.tile([P, P], BF16, tag="ident")
        make_identity(nc, ident[:])
        # selector for broadcasting the two per-head reciprocal rows to 64 partitions each
        sel2 = consts.tile([2, P], BF16, tag="sel2")
        nc.sync.dma_start(sel2[:], sel2_d[:])

        # --- resident activations ---
        qt_sb = resident.tile([P, S], BF16, tag="qt")      # QT [128f, 4096q]
        kt_sb = resident.tile([P, S], BF16, tag="kt")      # KT [128f, 4096k]
        # V per key tile: [128k, 130]: cols 0:64 = head0, col 64 = ones, 65:129 = head1, 129 = ones
        va_sb = resident.tile([P, NKT, 130], BF16, tag="va")
        nc.vector.memset(va_sb[:, :, 64:65], 1.0)
        nc.vector.memset(va_sb[:, :, 129:130], 1.0)

        # --- phase 1: projections ---
        with tc.tile_pool(name="xtp", bufs=4) as xtp, \
             tc.tile_pool(name="vts", bufs=2) as vts, \
             tc.tile_pool(name="pp", bufs=3, space="PSUM") as pp, \
             tc.tile_pool(name="tp", bufs=2, space="PSUM") as tpp:
            for qc in range(NQB):
                xts = []
                for h in range(NCH):
                    xt = xtp.tile([P, QB], BF16, tag="xt")
                    nc.sync.dma_start(xt[:], xt_d[h * P:(h + 1) * P, qc * QB:(qc + 1) * QB])
                    xts.append(xt)
                for (w_sb, dst) in ((wq_sb, qt_sb), (wk_sb, kt_sb)):
                    ps = pp.tile([P, QB], F32, tag="pp")
                    for h in range(NCH):
                        nc.tensor.matmul(ps[:], w_sb[:, h, :], xts[h][:],
                                         start=(h == 0), stop=(h == NCH - 1))
                    nc.vector.tensor_copy(dst[:, qc * QB:(qc + 1) * QB], ps[:])
                # V^T [128d, 512k] then PE-transpose to natural layout
                vt_ps = pp.tile([P, QB], F32, tag="pp")
                for h in range(NCH):
                    nc.tensor.matmul(vt_ps[:], wv_sb[:, h, :], xts[h][:],
                                     start=(h == 0), stop=(h == NCH - 1))
                vt_sb = vts.tile([P, QB], BF16, tag="vt")
                nc.vector.tensor_copy(vt_sb[:], vt_ps[:])
                for j in range(QB // P):
                    kt_idx = qc * (QB // P) + j
                    t_ps = tpp.tile([P, P], BF16, tag="tp")
                    nc.tensor.transpose(t_ps[:], vt_sb[:, j * P:(j + 1) * P], ident[:])
                    nc.vector.tensor_copy(va_sb[:, kt_idx, 0:HD], t_ps[:, 0:HD])
                    nc.vector.tensor_copy(va_sb[:, kt_idx, 65:65 + HD], t_ps[:, HD:P])

        # --- phase 2: attention + out-projection ---
        with tc.tile_pool(name="ep", bufs=3) as ep, \
             tc.tile_pool(name="cxs", bufs=3) as cxs, \
             tc.tile_pool(name="rcp", bufs=2) as rcp, \
             tc.tile_pool(name="ctxn", bufs=2) as ctxnp, \
             tc.tile_pool(name="outs", bufs=3) as outs, \
             tc.tile_pool(name="scp", bufs=2, space="PSUM") as scp, \
             tc.tile_pool(name="cxp", bufs=3, space="PSUM") as cxp:
            for qc in range(NQB):
                cx = [cxp.tile([P, QB], F32, tag="cx", name=f"cx{qc}_{i}") for i in range(2)]
                for hh in range(2):
                    off = 65 * hh
                    fs = slice(hh * HD, (hh + 1) * HD)
                    q_rhs = qt_sb[fs, qc * QB:(qc + 1) * QB]
                    for g in range(NKT // 2):
                        sc = scp.tile([P, 2, QB], F32, tag="sc")
                        for j in range(2):
                            kt = 2 * g + j
                            nc.tensor.matmul(sc[:, j, :],
                                             kt_sb[fs, kt * P:(kt + 1) * P],
                                             q_rhs, start=True, stop=True)
                        et = ep.tile([P, 2, QB], BF16, tag="et")
                        nc.scalar.activation(et[:], sc[:],
                                             mybir.ActivationFunctionType.Exp,
                                             bias=0.0, scale=0.125)
                        for j in range(2):
                            kt = 2 * g + j
                            nc.tensor.matmul(cx[hh][0:65, :],
                                             va_sb[:, kt, off:off + 65],
                                             et[:, j, :],
                                             start=(g == 0 and j == 0),
                                             stop=(g == NKT // 2 - 1 and j == 1))
                # softmax denominators -> [2, 512] via tiny SBUF-to-SBUF DMAs (partition move)
                cx_sb = [cxs.tile([P, QB], F32, tag="cxs", name=f"cxsb{qc}_{i}") for i in range(2)]
                for hh in range(2):
                    nc.vector.tensor_copy(cx_sb[hh][0:65, :], cx[hh][0:65, :])
                r2pre = rcp.tile([2, QB], F32, tag="r2pre")
                nc.sync.dma_start(r2pre[0:1, :], cx_sb[0][64:65, :])
                nc.sync.dma_start(r2pre[1:2, :], cx_sb[1][64:65, :])
                rec2f = rcp.tile([2, QB], F32, tag="rec2f")
                nc.vector.reciprocal(rec2f[:], r2pre[:])
                rec2 = rcp.tile([2, QB], BF16, tag="rec2")
                nc.vector.tensor_copy(rec2[:], rec2f[:])
                rx_ps = cxp.tile([P, QB], F32, tag="cx")
                nc.tensor.matmul(rx_ps[:], sel2[:], rec2[:], start=True, stop=True)
                # normalized ctx^T [128f, 512q]; head1 rows moved 0:64 -> 64:128 via DMA
                ctxn = ctxnp.tile([P, QB], BF16, tag="ctxn")
                nc.vector.tensor_tensor(ctxn[0:HD, :], cx_sb[0][0:HD, :],
                                        rx_ps[0:HD, :], mybir.AluOpType.mult)
                h1s = ctxnp.tile([P, QB], BF16, tag="h1s")
                h1c = ctxnp.tile([HD, QB], BF16, tag="h1c")
                nc.vector.tensor_copy(h1c[:], cx_sb[1][0:HD, :])
                nc.sync.dma_start(h1s[HD:P, :], h1c[:])
                nc.vector.tensor_tensor(ctxn[HD:P, :], h1s[HD:P, :],
                                        rx_ps[HD:P, :], mybir.AluOpType.mult)
                # out-projection: out[q, :] += ctx @ wo^T for this 512-query block
                for i in range(QB // P):
                    op = scp.tile([P, 2, QB], F32, tag="sc")
                    lhsT = ctxn[:, i * P:(i + 1) * P]
                    for j in range(2):
                        nc.tensor.matmul(op[:, j, :], lhsT, wo_sb[:, j * QB:(j + 1) * QB],
                                         start=True, stop=True)
                    ot = outs.tile([P, 2, QB], F32, tag="ot")
                    nc.vector.tensor_copy(ot[:], op[:])
                    nc.sync.dma_start(out_d[qc * QB + i * P: qc * QB + (i + 1) * P, :],
                                      ot[:].rearrange("p a b -> p (a b)"))
    return nc


_NC_CACHE = {}


def _get_nc():
    if "nc" not in _NC_CACHE:
        _NC_CACHE["nc"] = _build_nc()
    return _NC_CACHE["nc"]


def _sel2_const():
    s = np.zeros((2, P), dtype=ml_dtypes.bfloat16)
    s[0, 0:HD] = 1.0
    s[1, HD:P] = 1.0
    return s


def _prep_inputs(inputs, Wq, bq, Wk, bk, Wv, bv, Wo, bo):
    x = np.asarray(inputs, dtype=np.float32).reshape(S, HID)
    xt = np.zeros((HC, S), dtype=ml_dtypes.bfloat16)
    xt[:HID] = x.T.astype(ml_dtypes.bfloat16)
    xt[HID] = 1.0
    in_maps = []
    for c in range(NCORES):
        sl = slice(c * P, (c + 1) * P)

        def wpad(W, b):
            wp = np.zeros((HC, P), dtype=ml_dtypes.bfloat16)
            wp[:HID] = np.asarray(W, dtype=np.float32)[sl].T.astype(ml_dtypes.bfloat16)
            wp[HID] = np.asarray(b, dtype=np.float32)[sl].astype(ml_dtypes.bfloat16)
            return wp

        in_maps.append({
            "xt": xt,
            "wq": wpad(Wq, bq),
            "wk": wpad(Wk, bk),
            "wv": wpad(Wv, bv),
            "wo": np.ascontiguousarray(np.asarray(Wo, dtype=np.float32)[:, sl].T).astype(ml_dtypes.bfloat16),
            "sel2": _sel2_const(),
        })
    return in_maps


def _run(inputs, Wq, bq, Wk, bk, Wv, bv, Wo, bo, trace=False, **kw):
    nc = _get_nc()
    in_maps = _prep_inputs(inputs, Wq, bq, Wk, bk, Wv, bv, Wo, bo)
    res = run_bass_kernel_spmd(nc, in_maps, list(range(NCORES)), trace=trace, **kw)
    parts = np.stack([np.asarray(res.results[i]["out"]) for i in range(NCORES)])
    out = parts.sum(axis=0) + np.asarray(bo, dtype=np.float32)
    return out.reshape(1, S, HID).astype(np.float32), res


def kernel(inputs, Wq, bq, Wk, bk, Wv, bv, Wo, bo):
    out, _ = _run(inputs, Wq, bq, Wk, bk, Wv, bv, Wo, bo, trace=False)
    return out


# revision 14
# speedup vs baseline: 1.3833x; 1.3833x over previous
"""Trainium2 Bass kernel: 16-head attention (S=4096, D=1024) sharded 2 heads/core over 8 cores.

Layout per core c (slice = c*128:(c+1)*128 of the hidden dim = heads 2c, 2c+1):
  - host passes xT_pad [1152, 4096]  (x.T padded: row 1024 = ones for bias fold, rest 0)
  - wq/wk/wv [1152, 128]: rows 0:1024 = W[slice].T, row 1024 = b[slice]
  - wo [128, 1024] = Wo[:, slice].T
  - device computes QT,KT [128f, 4096q], V [4096k, 128d], then per 512-query block:
    scoresT[k, q] = (K Q^T), exp (scale=1/8 folded in, no max-subtraction: scores ~ N(0,1)),
    PV with an appended ones-column in V giving softmax denominators, normalization via a
    broadcast-reciprocal matmul, then partial out-projection. Host sums the 8 partials.
"""

import os
import sys

import numpy as np
import ml_dtypes

if os.path.isdir("/opt/trn_rl_repo") and "/opt/trn_rl_repo" not in sys.path:
    sys.path.insert(0, "/opt/trn_rl_repo")

from contextlib import ExitStack

from concourse import bass, tile
from concourse.bass_utils import run_bass_kernel_spmd
from concourse.masks import make_identity

mybir = bass.mybir
F32 = mybir.dt.float32
F32R = mybir.dt.float32r
BF16 = mybir.dt.bfloat16

P = 128
S = 4096
HID = 1024
HC = 1152          # padded contraction: 9 chunks of 128 (chunk 8 carries the bias fold)
NCH = 9
NCORES = 8
QB = 512           # query block
NQB = S // QB      # 8
NKT = S // P       # 32 key tiles
HD = 64            # head dim; 2 local heads per core




def _split_multiwaits(bir_json):
    """Walrus in this toolchain encodes at most one semaphore wait per TPB
    instruction; hoist extra waits onto injected pure-wait EventSemaphore
    instructions immediately before, on the same engine."""
    import json as _json

    bir = _json.loads(bir_json)
    n = [0]
    for fn in bir["functions"]:
        for blk in fn["blocks"]:
            out = []
            for ins in blk["instructions"]:
                si = ins.get("sync_info") or {}
                waits = si.get("on_wait") or []
                if len(waits) > 1 and ins.get("opcode") != "EventSemaphore":
                    for w in waits[:-1]:
                        n[0] += 1
                        out.append({
                            "debug": ins.get("debug", 0),
                            "engine": ins["engine"],
                            "ins": [],
                            "name": f"{ins['name']}_sw{n[0]}",
                            "opcode": "EventSemaphore",
                            "outs": [],
                            "sync_info": {"on_update": [], "on_wait": [w]},
                        })
                    si["on_wait"] = [waits[-1]]
                out.append(ins)
            blk["instructions"] = out
    return _json.dumps(bir).encode()


def _install_compile_patch():
    from concourse import bass_utils as _bu
    from concourse import bass2jax as _b2j

    if getattr(_bu, "_ant_waitsplit", False):
        return
    _orig = _bu.compile_bir_kernel

    def _patched(bir_json, tmpdir, neff_name="file.neff"):
        return _orig(_split_multiwaits(bir_json), tmpdir, neff_name)

    _bu.compile_bir_kernel = _patched
    _b2j.compile_bir_kernel = _patched
    _bu._ant_waitsplit = True


_install_compile_patch()


def _build_nc():
    nc = bass.Bass()
    xt_d = nc.declare_dram_parameter("xt", [HC, S], BF16, isOutput=False)
    wq_d = nc.declare_dram_parameter("wq", [HC, P], BF16, isOutput=False)
    wk_d = nc.declare_dram_parameter("wk", [HC, P], BF16, isOutput=False)
    wv_d = nc.declare_dram_parameter("wv", [HC, P], BF16, isOutput=False)
    wo_d = nc.declare_dram_parameter("wo", [P, HID], BF16, isOutput=False)
    sel2_d = nc.declare_dram_parameter("sel2", [2, P], BF16, isOutput=False)
    out_d = nc.declare_dram_parameter("out", [S, HID], F32, isOutput=True)

    with tile.TileContext(nc) as tc, ExitStack() as ctx:
        consts = ctx.enter_context(tc.tile_pool(name="consts", bufs=1))
        resident = ctx.enter_context(tc.tile_pool(name="resident", bufs=1))

        # --- constants ---
        wq_sb = consts.tile([P, NCH, P], BF16, tag="wq")
        wk_sb = consts.tile([P, NCH, P], BF16, tag="wk")
        wv_sb = consts.tile([P, NCH, P], BF16, tag="wv")
        nc.sync.dma_start(wq_sb[:], wq_d.rearrange("(c p) m -> p c m", p=P))
        nc.sync.dma_start(wk_sb[:], wk_d.rearrange("(c p) m -> p c m", p=P))
        nc.sync.dma_start(wv_sb[:], wv_d.rearrange("(c p) m -> p c m", p=P))
        wo_sb = consts.tile([P, HID], BF16, tag="wo")
        nc.sync.dma_start(wo_sb[:], wo_d[:])
        ident = consts.tile([P, P], BF16, tag="ident")
        make_identity(nc, ident[:])
        # selector for broadcasting the two per-head reciprocal rows to 64 partitions each
        sel2 = consts.tile([2, P], BF16, tag="sel2")
        nc.sync.dma_start(sel2[:], sel2_d[:])

        # --- resident activations ---
        qt_sb = resident.tile([P, S], BF16, tag="qt")      # QT [128f, 4096q]
        kt_sb = resident.tile([P, S], BF16, tag="kt")      # KT [128f, 4096k]
        # V per key tile: [128k, 130]: cols 0:64 = head0, col 64 = ones, 65:129 = head1, 129 = ones
        va_sb = resident.tile([P, NKT, 130], BF16, tag="va")
        nc.vector.memset(va_sb[:, :, 64:65], 1.0)
        nc.vector.memset(va_sb[:, :, 129:130], 1.0)

        # --- phase 1: projections ---
        with tc.tile_pool(name="xtp", bufs=4) as xtp, \
             tc.tile_pool(name="vts", bufs=2) as vts, \
             tc.tile_pool(name="pp", bufs=3, space="PSUM") as pp, \
             tc.tile_pool(name="tp", bufs=2, space="PSUM") as tpp:
            for qc in range(NQB):
                xts = []
                for h in range(NCH):
                    xt = xtp.tile([P, QB], BF16, tag="xt")
                    nc.sync.dma_start(xt[:], xt_d[h * P:(h + 1) * P, qc * QB:(qc + 1) * QB])
                    xts.append(xt)
                for (w_sb, dst) in ((wq_sb, qt_sb), (wk_sb, kt_sb)):
                    ps = pp.tile([P, QB], F32, tag="pp")
                    for h in range(NCH):
                        nc.tensor.matmul(ps[:], w_sb[:, h, :], xts[h][:],
                                         start=(h == 0), stop=(h == NCH - 1))
                    nc.vector.tensor_copy(dst[:, qc * QB:(qc + 1) * QB], ps[:])
                # V^T [128d, 512k] then PE-transpose to natural layout
                vt_ps = pp.tile([P, QB], F32, tag="pp")
                for h in range(NCH):
                    nc.tensor.matmul(vt_ps[:], wv_sb[:, h, :], xts[h][:],
                                     start=(h == 0), stop=(h == NCH - 1))
                vt_sb = vts.tile([P, QB], BF16, tag="vt")
                nc.vector.tensor_copy(vt_sb[:], vt_ps[:])
                for j in range(QB // P):
                    kt_idx = qc * (QB // P) + j
                    t_ps = tpp.tile([P, P], BF16, tag="tp")
                    nc.tensor.transpose(t_ps[:], vt_sb[:, j * P:(j + 1) * P], ident[:])
                    nc.vector.tensor_copy(va_sb[:, kt_idx, 0:HD], t_ps[:, 0:HD])
                    nc.vector.tensor_copy(va_sb[:, kt_idx, 65:65 + HD], t_ps[:, HD:P])

        # --- phase 2: attention + out-projection ---
        with tc.tile_pool(name="ep", bufs=3) as ep, \
             tc.tile_pool(name="cxs", bufs=3) as cxs, \
             tc.tile_pool(name="rcp", bufs=2) as rcp, \
             tc.tile_pool(name="ctxn", bufs=2) as ctxnp, \
             tc.tile_pool(name="outs", bufs=3) as outs, \
             tc.tile_pool(name="scp", bufs=3, space="PSUM") as scp, \
             tc.tile_pool(name="cxp", bufs=2, space="PSUM") as cxp:
            for qc in range(NQB):
                cx = [cxp.tile([P, QB], F32, tag="cx", name=f"cx{qc}_{i}") for i in range(2)]
                for g in range(NKT // 2):
                    for hh in range(2):
                        off = 65 * hh
                        fs = slice(hh * HD, (hh + 1) * HD)
                        q_rhs = qt_sb[fs, qc * QB:(qc + 1) * QB]
                        sc = scp.tile([P, 2, QB], F32, tag="sc",
                                      name=f"sc{qc}_{g}_{hh}")
                        for j in range(2):
                            kt = 2 * g + j
                            nc.tensor.matmul(sc[:, j, :],
                                             kt_sb[fs, kt * P:(kt + 1) * P],
                                             q_rhs, start=True, stop=True)
                        et = ep.tile([P, 2, QB], BF16, tag="et",
                                     name=f"et{qc}_{g}_{hh}")
                        nc.scalar.activation(et[:], sc[:],
                                             mybir.ActivationFunctionType.Exp,
                                             bias=0.0, scale=0.125)
                        for j in range(2):
                            kt = 2 * g + j
                            nc.tensor.matmul(cx[hh][0:65, :],
                                             va_sb[:, kt, off:off + 65],
                                             et[:, j, :],
                                             start=(g == 0 and j == 0),
                                             stop=(g == NKT // 2 - 1 and j == 1))
                # softmax denominators -> [2, 512] via tiny SBUF-to-SBUF DMAs (partition move)
                cx_sb = [cxs.tile([P, QB], F32, tag="cxs", name=f"cxsb{qc}_{i}") for i in range(2)]
                for hh in range(2):
                    nc.vector.tensor_copy(cx_sb[hh][0:65, :], cx[hh][0:65, :])
                r2pre = rcp.tile([2, QB], F32, tag="r2pre")
                nc.sync.dma_start(r2pre[0:1, :], cx_sb[0][64:65, :])
                nc.sync.dma_start(r2pre[1:2, :], cx_sb[1][64:65, :])
                rec2f = rcp.tile([2, QB], F32, tag="rec2f")
                nc.vector.reciprocal(rec2f[:], r2pre[:])
                rec2 = rcp.tile([2, QB], BF16, tag="rec2")
                nc.vector.tensor_copy(rec2[:], rec2f[:])
                rx_ps = scp.tile([P, QB], F32, tag="sc")
                nc.tensor.matmul(rx_ps[:], sel2[:], rec2[:], start=True, stop=True)
                # normalized ctx^T [128f, 512q]; head1 rows moved 0:64 -> 64:128 via DMA
                ctxn = ctxnp.tile([P, QB], BF16, tag="ctxn")
                nc.vector.tensor_tensor(ctxn[0:HD, :], cx_sb[0][0:HD, :],
                                        rx_ps[0:HD, :], mybir.AluOpType.mult)
                h1s = ctxnp.tile([P, QB], BF16, tag="h1s")
                h1c = ctxnp.tile([HD, QB], BF16, tag="h1c")
                nc.vector.tensor_copy(h1c[:], cx_sb[1][0:HD, :])
                nc.sync.dma_start(h1s[HD:P, :], h1c[:])
                nc.vector.tensor_tensor(ctxn[HD:P, :], h1s[HD:P, :],
                                        rx_ps[HD:P, :], mybir.AluOpType.mult)
                # out-projection: out[q, :] += ctx @ wo^T for this 512-query block
                for i in range(QB // P):
                    op = scp.tile([P, 2, QB], F32, tag="sc")
                    lhsT = ctxn[:, i * P:(i + 1) * P]
                    for j in range(2):
                        nc.tensor.matmul(op[:, j, :], lhsT, wo_sb[:, j * QB:(j + 1) * QB],
                                         start=True, stop=True)
                    ot = outs.tile([P, 2, QB], F32, tag="ot")
                    nc.vector.tensor_copy(ot[:], op[:])
                    nc.sync.dma_start(out_d[qc * QB + i * P: qc * QB + (i + 1) * P, :],
                                      ot[:].rearrange("p a b -> p (a b)"))
    return nc


_NC_CACHE = {}


def _get_nc():
    if "nc" not in _NC_CACHE:
        _NC_CACHE["nc"] = _build_nc()
    return _NC_CACHE["nc"]


def _sel2_const():
    s = np.zeros((2, P), dtype=ml_dtypes.bfloat16)
    s[0, 0:HD] = 1.0
    s[1, HD:P] = 1.0
    return s


def _prep_inputs(inputs, Wq, bq, Wk, bk, Wv, bv, Wo, bo):
    x = np.asarray(inputs, dtype=np.float32).reshape(S, HID)
    xt = np.zeros((HC, S), dtype=ml_dtypes.bfloat16)
    xt[:HID] = x.T.astype(ml_dtypes.bfloat16)
    xt[HID] = 1.0
    in_maps = []
    for c in range(NCORES):
        sl = slice(c * P, (c + 1) * P)

        def wpad(W, b):
            wp = np.zeros((HC, P), dtype=ml_dtypes.bfloat16)
            wp[:HID] = np.asarray(W, dtype=np.float32)[sl].T.astype(ml_dtypes.bfloat16)
            wp[HID] = np.asarray(b, dtype=np.float32)[sl].astype(ml_dtypes.bfloat16)
            return wp

        in_maps.append({
            "xt": xt,
            "wq": wpad(Wq, bq),
            "wk": wpad(Wk, bk),
            "wv": wpad(Wv, bv),
            "wo": np.ascontiguousarray(np.asarray(Wo, dtype=np.float32)[:, sl].T).astype(ml_dtypes.bfloat16),
            "sel2": _sel2_const(),
        })
    return in_maps


def _run(inputs, Wq, bq, Wk, bk, Wv, bv, Wo, bo, trace=False, **kw):
    nc = _get_nc()
    in_maps = _prep_inputs(inputs, Wq, bq, Wk, bk, Wv, bv, Wo, bo)
    res = run_bass_kernel_spmd(nc, in_maps, list(range(NCORES)), trace=trace, **kw)
    parts = np.stack([np.asarray(res.results[i]["out"]) for i in range(NCORES)])
    out = parts.sum(axis=0) + np.asarray(bo, dtype=np.float32)
    return out.reshape(1, S, HID).astype(np.float32), res


def kernel(inputs, Wq, bq, Wk, bk, Wv, bv, Wo, bo):
    out, _ = _run(inputs, Wq, bq, Wk, bk, Wv, bv, Wo, bo, trace=False)
    return out
